# revision 31
# baseline (speedup 1.0000x reference)
"""Trainium2 Bass kernel for grouped per-channel linears (nn_GroupedLinearsAdvanced).

Math: out[b, o, d] = sum_i x[b, i, d] * W[d, i, o] + bias[d, o]
with x: [16, 128, 4096] f32, W: [4096, 128, 128] f32, bias: [4096, 128] f32,
out: [16, 128, 4096] f32.

Sharding: channel dim D=4096 split into 8 contiguous slabs of 512 channels,
one per NeuronCore; x slices replicated per-slab, no cross-device reduction.

Per-core dataflow (DMA-bound; ~20 MB of HBM traffic per core at ~26 GB/s
per SDMA engine x 16 engines):
  - host pre-permutes inputs so every DMA moves long contiguous
    per-partition runs; x and W are cast to bf16 on host (halves W traffic
    vs fp32; rel-err ~3e-3 against the fp32 reference),
  - x slab resident in SBUF: layout [i, dl*16+b]; both 1 MB chunks on the
    scalar HWDGE ring,
  - the whole 16 MB W slab is prefetched through SBUF in 8 x 2 MB tiles,
    all dma_starts issued up front, tiles alternating between the sync and
    scalar HWDGE rings (SDMA engines alternate descriptors between rings;
    a single ring cannot sustain line rate),
  - per channel: one matmul  PS[o, b] = W_d.T @ x_d.T  (lhsT = W_d),
    32 channels accumulate side-by-side into one 512-f32 PSUM bank,
  - bias (when nonzero) seeds each PSUM bank via a bf16 one-hot expansion
    matmul: PS[o, j*16+b] = bias[g*32+j, o] = (BN_g).T @ E; skipped when
    the supplied bias is identically zero,
  - DVE evacuates each bank to SBUF casting f32 -> bf16 (halves out DMA),
  - outs stream to HBM via the GPSIMD SWDGE queue so they never queue in
    a HWDGE ring FIFO behind undrained W packets; host casts back to f32.

MM_DTYPE picks the tensor-engine path for x/W:
  "f32"  — exact fp32 (hardware runs 2 half-speed passes per matmul),
  "f32r" — same fp32 bytes, single-pass reduced-precision PE mode,
  "bf16" — host-side cast, halves DMA traffic, single-pass matmuls + FWL,
  "mix3" — bf16 hi+lo split, fp32-class accuracy at fp32 bytes (slow path).
"""

import ml_dtypes
import numpy as np

from concourse import bacc, mybir, tile
from concourse.bass_utils import run_bass_kernel_spmd

B = 16           # batch
IN_D = 128       # contraction dim (SBUF partitions)
OUT_D = 128      # per-channel output dim
D_TOTAL = 4096   # channels
NCORES = 8
D_C = D_TOTAL // NCORES      # 512 channels per core
BANK_CH = 32                 # channels per PSUM bank (32*16 = 512 fp32 = 1 bank)
N_BANKS = D_C // BANK_CH     # 16

X_COLS = D_C * B                 # 8192
BN_COLS = N_BANKS * OUT_D        # 2048
EH_COLS = BANK_CH * B            # 512
CB_COLS = BN_COLS + EH_COLS      # bias + one-hot constant tensor

F32 = mybir.dt.float32
BF16 = mybir.dt.bfloat16

MM_DTYPE = "bf16"

_DT = {
    "f32": (F32, np.float32),
    "f32r": (mybir.dt.float32r, np.float32),
    "bf16": (BF16, ml_dtypes.bfloat16),
    # mix3: W and x split into bf16 hi+lo parts; 3 single-pass matmuls
    # per channel (hi*hi + lo*hi + hi*lo) recover ~1e-5 accuracy while
    # keeping bf16 tensor-engine throughput. Same HBM bytes as fp32.
    "mix3": (BF16, ml_dtypes.bfloat16),
}

_cached = {}


def _build(mode, has_bias):
    dt_mm, _ = _DT[mode]
    nparts = 2 if mode == "mix3" else 1  # hi/lo operand copies
    out_dt = BF16 if mode == "bf16" else F32
    nc = bacc.Bacc()
    xc = nc.dram_tensor("xc", [IN_D, nparts * X_COLS], dt_mm, kind="ExternalInput")
    wr = nc.dram_tensor(
        "wr", [IN_D, nparts * D_C * OUT_D], dt_mm, kind="ExternalInput"
    )
    if has_bias:
        cb = nc.dram_tensor("cb", [BANK_CH, CB_COLS], BF16, kind="ExternalInput")
    outr = nc.dram_tensor("outr", [OUT_D, D_C * B], out_dt, kind="ExternalOutput")

    # Streaming-path (non-prefetch) tile size; the prefetch path below uses
    # `sizes` instead.
    tile_ch = 32 // nparts
    n_tiles = D_C // tile_ch
    banks_per_tile = max(1, tile_ch // BANK_CH)
    wcols_per_ch = nparts * OUT_D
    # Prefetch the whole W slab when it fits in SBUF (bf16: 8 tiles x
    # 16 KB/partition = 128 KB). All W dma_starts issue up front, so a
    # later out-DMA's semaphore wait on the sync ring can never delay a
    # W transfer (HWDGE rings are FIFO per issuing engine).
    w_kb_per_part = n_tiles * tile_ch * wcols_per_ch * (2 if dt_mm != F32 else 4)
    prefetch_all = w_kb_per_part <= 128 * 1024
    # Uniform 64-channel (2 MB) W tiles. Finer or variable tail tiles were
    # tried and REGRESSED: at stream end the per-transfer completion
    # semaphores fire 1-3 us after the data (3-way ring contention for the
    # final sem descriptors), and the in-order PE serializes those waits —
    # fewer, uniform transfers win. Splitting the final bank's cast/store
    # also regressed (extra tail dependencies).
    sizes = [BANK_CH] * N_BANKS
    with tile.TileContext(nc) as tc:
        with (
            tc.tile_pool(name="xp", bufs=1) as xp,
            tc.tile_pool(name="wpa", bufs=len(sizes)) as wpa,
            tc.tile_pool(name="op", bufs=N_BANKS) as op,
            tc.tile_pool(name="pp", bufs=8, space="PSUM") as pp,
        ):
            wp_by_size = {BANK_CH: wpa}
            XC = xp.tile([IN_D, nparts * X_COLS], dt_mm)
            # Chunks so early banks can start before the back half lands;
            # chunk-major order so bank 0 gets hi AND lo slices first.
            # One chunk per ring keeps ring byte loads equal.
            # x rides the SWDGE queue: the Q7 finishes its preamble ~1.5 us
            # before the HWDGE sequencers reach their first dma_start, so x
            # descriptors hit the engines earlier and the stream starts
            # sooner. It also keeps both HWDGE rings at exactly 8 MB of W
            # each (engines alternate descriptors per ring, so unequal ring
            # bytes let one ring's tail pace the kernel — trace-verified).
            half = X_COLS // 2
            for ch in range(2):
                for p in range(nparts):
                    lo = p * X_COLS + ch * half
                    xeng = nc.gpsimd if prefetch_all else nc.scalar
                    xeng.dma_start(
                        XC[:, lo:lo + half], xc[:, lo:lo + half]
                    )
            if has_bias:
                CB = xp.tile([BANK_CH, CB_COLS], BF16)
                nc.scalar.dma_start(CB[:], cb[:])

            if prefetch_all:
                # W alternates across both HWDGE rings (one ring alone cannot
                # sustain full line rate; SDMA engines alternate descriptors
                # between rings, so both rings must carry work). Outs go to
                # the GPSIMD SWDGE queue: in a HWDGE ring they would sit in
                # FIFO order behind the remaining W packets and only drain
                # after the whole W stream.
                tiles = []
                c0 = 0
                for i, tch in enumerate(sizes):
                    WT = wp_by_size[tch].tile([IN_D, tch * OUT_D], dt_mm)
                    weng = nc.sync if i % 2 == 0 else nc.scalar
                    weng.dma_start(
                        WT[:], wr[:, c0 * OUT_D:(c0 + tch) * OUT_D]
                    )
                    tiles.append((c0, c0 + tch, WT))
                    c0 += tch

                def w_slice(dl):
                    for a, b2, WT in tiles:
                        if a <= dl < b2:
                            return WT[:, (dl - a) * OUT_D:(dl - a + 1) * OUT_D]

                for g in range(N_BANKS):
                    PS = pp.tile([OUT_D, BANK_CH * B], F32)
                    if has_bias:
                        nc.tensor.matmul(
                            PS[:],
                            CB[:, g * OUT_D:(g + 1) * OUT_D],
                            CB[:, BN_COLS:CB_COLS],
                            start=True,
                            stop=False,
                        )
                    for j in range(BANK_CH):
                        dl = g * BANK_CH + j
                        nc.tensor.matmul(
                            PS[:, j * B:(j + 1) * B],
                            w_slice(dl),
                            XC[:, dl * B:(dl + 1) * B],
                            start=(not has_bias) and j == 0,
                            stop=(j == BANK_CH - 1),
                        )
                    OB = op.tile([OUT_D, BANK_CH * B], out_dt)
                    base = g * BANK_CH * B
                    nc.vector.tensor_copy(OB[:], PS[:])
                    # All outs on SWDGE, including the last ones: routing the
                    # final outs through the (idle) HWDGE rings was tried and
                    # regressed ~6 us.
                    nc.gpsimd.dma_start(
                        outr[:, base:base + BANK_CH * B], OB[:]
                    )
            else:
                nch = tile_ch * wcols_per_ch
                for t in range(n_tiles):
                    WT = wpa.tile([IN_D, nch], dt_mm)
                    # Alternate the two HWDGE rings so W transfers overlap.
                    weng = nc.sync if t % 2 == 0 else nc.scalar
                    weng.dma_start(WT[:], wr[:, t * nch:(t + 1) * nch])
                    for h in range(banks_per_tile):
                        g = t * banks_per_tile + h
                        PS = pp.tile([OUT_D, BANK_CH * B], F32)
                        if has_bias:
                            # Seed bank: PS[o, j*16+b] = bias[g*32+j, o].
                            nc.tensor.matmul(
                                PS[:],
                                CB[:, g * OUT_D:(g + 1) * OUT_D],
                                CB[:, BN_COLS:CB_COLS],
                                start=True,
                                stop=False,
                            )
                        for j in range(BANK_CH):
                            jt = h * BANK_CH + j
                            dl = g * BANK_CH + j
                            out_sl = PS[:, j * B:(j + 1) * B]
                            whi = WT[
                                :, jt * wcols_per_ch:jt * wcols_per_ch + OUT_D
                            ]
                            xhi = XC[:, dl * B:(dl + 1) * B]
                            nc.tensor.matmul(
                                out_sl,
                                whi,
                                xhi,
                                start=(not has_bias) and j == 0,
                                stop=(mode != "mix3") and (j == BANK_CH - 1),
                            )
                            if mode == "mix3":
                                wlo = WT[
                                    :,
                                    jt * wcols_per_ch + OUT_D:
                                    (jt + 1) * wcols_per_ch,
                                ]
                                xlo = XC[
                                    :, X_COLS + dl * B:X_COLS + (dl + 1) * B
                                ]
                                nc.tensor.matmul(
                                    out_sl, whi, xlo, start=False, stop=False
                                )
                                nc.tensor.matmul(
                                    out_sl,
                                    wlo,
                                    xhi,
                                    start=False,
                                    stop=(j == BANK_CH - 1),
                                )
                        OB = op.tile([OUT_D, BANK_CH * B], out_dt)
                        nc.vector.tensor_copy(OB[:], PS[:])
                        nc.sync.dma_start(
                            outr[:, g * BANK_CH * B:(g + 1) * BANK_CH * B],
                            OB[:],
                        )

    nc.finalize()
    return nc


def _pack_x(x, sl):
    # [b, i, dslab] -> [i, dl*16+b]
    return np.ascontiguousarray(x[:, :, sl].transpose(1, 2, 0)).reshape(
        IN_D, X_COLS
    )


def _pack_bias(b, sl, eh):
    bnr = np.ascontiguousarray(
        b[sl].reshape(N_BANKS, BANK_CH, OUT_D).transpose(1, 0, 2)
    ).reshape(BANK_CH, BN_COLS)
    cbv = np.zeros((BANK_CH, CB_COLS), dtype=ml_dtypes.bfloat16)
    cbv[:, :BN_COLS] = bnr.astype(ml_dtypes.bfloat16)
    cbv[:, BN_COLS:] = eh.astype(ml_dtypes.bfloat16)
    return cbv


def _prep_core_inputs(x, W, b, mode, has_bias):
    _, np_mm = _DT[mode]
    eh = np.repeat(np.eye(BANK_CH, dtype=np.float32), B, axis=1)
    if mode == "mix3":
        bf = ml_dtypes.bfloat16
        xh = x.astype(bf)
        xl = (x - xh.astype(np.float32)).astype(bf)
        Wh = W.astype(bf)
        Wl = (W - Wh.astype(np.float32)).astype(bf)
    in_maps = []
    for c in range(NCORES):
        sl = slice(c * D_C, (c + 1) * D_C)
        if mode == "mix3":
            xcv = np.concatenate(
                [_pack_x(xh.astype(np.float32), sl), _pack_x(xl.astype(np.float32), sl)],
                axis=1,
            ).astype(bf)
            wrv = np.ascontiguousarray(
                np.stack(
                    [Wh[sl].transpose(1, 0, 2), Wl[sl].transpose(1, 0, 2)],
                    axis=2,
                )
            ).reshape(IN_D, D_C * 2 * OUT_D)
            m = {"xc": xcv, "wr": wrv}
            if has_bias:
                m["cb"] = _pack_bias(b, sl, eh)
            in_maps.append(m)
            continue
        xr = _pack_x(x, sl).astype(np_mm, copy=False)
        wrv = (
            np.ascontiguousarray(W[sl].transpose(1, 0, 2))
            .reshape(IN_D, D_C * OUT_D)
            .astype(np_mm, copy=False)
        )
        m = {"xc": xr, "wr": wrv}
        if has_bias:
            m["cb"] = _pack_bias(b, sl, eh)
        in_maps.append(m)
    return in_maps


def run(inputs, trace=False, mode=None):
    mode = mode or MM_DTYPE
    x = np.asarray(inputs["x"], dtype=np.float32)
    W = np.asarray(inputs["W"], dtype=np.float32)
    b = np.asarray(inputs["b"], dtype=np.float32)
    has_bias = bool(np.any(b))
    key = (mode, has_bias)
    if key not in _cached:
        _cached[key] = _build(mode, has_bias)
    in_maps = _prep_core_inputs(x, W, b, mode, has_bias)
    res = run_bass_kernel_spmd(
        _cached[key], in_maps, core_ids=list(range(NCORES)), trace=trace
    )
    out = np.empty((B, OUT_D, D_TOTAL), dtype=np.float32)
    for c in range(NCORES):
        sl = slice(c * D_C, (c + 1) * D_C)
        out[:, :, sl] = (
            np.asarray(res.results[c]["outr"])
            .astype(np.float32)
            .reshape(OUT_D, D_C, B)
            .transpose(2, 0, 1)
        )
    return out, res


def kernel(**inputs):
    out, _ = run(inputs)
    return out



# revision 32
# speedup vs baseline: 1.0196x; 1.0196x over previous
"""Trainium2 Bass kernel for grouped per-channel linears (nn_GroupedLinearsAdvanced).

Math: out[b, o, d] = sum_i x[b, i, d] * W[d, i, o] + bias[d, o]
with x: [16, 128, 4096] f32, W: [4096, 128, 128] f32, bias: [4096, 128] f32,
out: [16, 128, 4096] f32.

Sharding: channel dim D=4096 split into 8 contiguous slabs of 512 channels,
one per NeuronCore; x slices replicated per-slab, no cross-device reduction.

Per-core dataflow (DMA-bound; ~20 MB of HBM traffic per core at ~26 GB/s
per SDMA engine x 16 engines):
  - host pre-permutes inputs so every DMA moves long contiguous
    per-partition runs; x and W are cast to bf16 on host (halves W traffic
    vs fp32; rel-err ~3e-3 against the fp32 reference),
  - x slab resident in SBUF: layout [i, dl*16+b]; both 1 MB chunks on the
    scalar HWDGE ring,
  - the whole 16 MB W slab is prefetched through SBUF in 8 x 2 MB tiles,
    all dma_starts issued up front, tiles alternating between the sync and
    scalar HWDGE rings (SDMA engines alternate descriptors between rings;
    a single ring cannot sustain line rate),
  - per channel: one matmul  PS[o, b] = W_d.T @ x_d.T  (lhsT = W_d),
    32 channels accumulate side-by-side into one 512-f32 PSUM bank,
  - bias (when nonzero) seeds each PSUM bank via a bf16 one-hot expansion
    matmul: PS[o, j*16+b] = bias[g*32+j, o] = (BN_g).T @ E; skipped when
    the supplied bias is identically zero,
  - DVE evacuates each bank to SBUF casting f32 -> bf16 (halves out DMA),
  - outs stream to HBM via the GPSIMD SWDGE queue so they never queue in
    a HWDGE ring FIFO behind undrained W packets; host casts back to f32.

MM_DTYPE picks the tensor-engine path for x/W:
  "f32"  — exact fp32 (hardware runs 2 half-speed passes per matmul),
  "f32r" — same fp32 bytes, single-pass reduced-precision PE mode,
  "bf16" — host-side cast, halves DMA traffic, single-pass matmuls + FWL,
  "mix3" — bf16 hi+lo split, fp32-class accuracy at fp32 bytes (slow path).
"""

import ml_dtypes
import numpy as np

from concourse import bacc, mybir, tile
from concourse.bass_utils import run_bass_kernel_spmd

B = 16           # batch
IN_D = 128       # contraction dim (SBUF partitions)
OUT_D = 128      # per-channel output dim
D_TOTAL = 4096   # channels
NCORES = 8
D_C = D_TOTAL // NCORES      # 512 channels per core
BANK_CH = 32                 # channels per PSUM bank (32*16 = 512 fp32 = 1 bank)
N_BANKS = D_C // BANK_CH     # 16

X_COLS = D_C * B                 # 8192
BN_COLS = N_BANKS * OUT_D        # 2048
EH_COLS = BANK_CH * B            # 512
CB_COLS = BN_COLS + EH_COLS      # bias + one-hot constant tensor

F32 = mybir.dt.float32
BF16 = mybir.dt.bfloat16

MM_DTYPE = "bf16"

_DT = {
    "f32": (F32, np.float32),
    "f32r": (mybir.dt.float32r, np.float32),
    "bf16": (BF16, ml_dtypes.bfloat16),
    # mix3: W and x split into bf16 hi+lo parts; 3 single-pass matmuls
    # per channel (hi*hi + lo*hi + hi*lo) recover ~1e-5 accuracy while
    # keeping bf16 tensor-engine throughput. Same HBM bytes as fp32.
    "mix3": (BF16, ml_dtypes.bfloat16),
}

_cached = {}


def _build(mode, has_bias):
    dt_mm, _ = _DT[mode]
    nparts = 2 if mode == "mix3" else 1  # hi/lo operand copies
    out_dt = BF16 if mode == "bf16" else F32
    nc = bacc.Bacc()
    xc = nc.dram_tensor("xc", [IN_D, nparts * X_COLS], dt_mm, kind="ExternalInput")
    wr = nc.dram_tensor(
        "wr", [IN_D, nparts * D_C * OUT_D], dt_mm, kind="ExternalInput"
    )
    if has_bias:
        cb = nc.dram_tensor("cb", [BANK_CH, CB_COLS], BF16, kind="ExternalInput")
    outr = nc.dram_tensor("outr", [OUT_D, D_C * B], out_dt, kind="ExternalOutput")

    # Streaming-path (non-prefetch) tile size; the prefetch path below uses
    # `sizes` instead.
    tile_ch = 32 // nparts
    n_tiles = D_C // tile_ch
    banks_per_tile = max(1, tile_ch // BANK_CH)
    wcols_per_ch = nparts * OUT_D
    # Prefetch the whole W slab when it fits in SBUF (bf16: 8 tiles x
    # 16 KB/partition = 128 KB). All W dma_starts issue up front, so a
    # later out-DMA's semaphore wait on the sync ring can never delay a
    # W transfer (HWDGE rings are FIFO per issuing engine).
    w_kb_per_part = n_tiles * tile_ch * wcols_per_ch * (2 if dt_mm != F32 else 4)
    prefetch_all = w_kb_per_part <= 128 * 1024
    # Uniform 64-channel (2 MB) W tiles. Finer or variable tail tiles were
    # tried and REGRESSED: at stream end the per-transfer completion
    # semaphores fire 1-3 us after the data (3-way ring contention for the
    # final sem descriptors), and the in-order PE serializes those waits —
    # fewer, uniform transfers win. Splitting the final bank's cast/store
    # also regressed (extra tail dependencies).
    sizes = [BANK_CH] * N_BANKS
    with tile.TileContext(nc) as tc:
        with (
            tc.tile_pool(name="xp", bufs=1) as xp,
            tc.tile_pool(name="wpa", bufs=len(sizes)) as wpa,
            tc.tile_pool(name="op", bufs=N_BANKS) as op,
            tc.tile_pool(name="pp", bufs=8, space="PSUM") as pp,
        ):
            wp_by_size = {BANK_CH: wpa}
            XC = xp.tile([IN_D, nparts * X_COLS], dt_mm)
            # Chunks so early banks can start before the back half lands;
            # chunk-major order so bank 0 gets hi AND lo slices first.
            # One chunk per ring keeps ring byte loads equal.
            # One chunk per HWDGE ring: with both chunks on one ring that
            # ring carries 2 MB more W+x than the other and its final tiles
            # land ~9 us later, pacing the whole tail (trace-verified).
            # Loading x via the SWDGE queue instead was tried and regressed
            # ~5 us (every matmul waits on x; SWDGE completion is slower).
            half = X_COLS // 2
            for ch in range(2):
                for p in range(nparts):
                    lo = p * X_COLS + ch * half
                    xeng = nc.sync if (ch + p) % 2 == 0 else nc.scalar
                    xeng.dma_start(
                        XC[:, lo:lo + half], xc[:, lo:lo + half]
                    )
            if has_bias:
                CB = xp.tile([BANK_CH, CB_COLS], BF16)
                nc.scalar.dma_start(CB[:], cb[:])

            if prefetch_all:
                # W alternates across both HWDGE rings (one ring alone cannot
                # sustain full line rate; SDMA engines alternate descriptors
                # between rings, so both rings must carry work). Outs go to
                # the GPSIMD SWDGE queue: in a HWDGE ring they would sit in
                # FIFO order behind the remaining W packets and only drain
                # after the whole W stream.
                tiles = []
                c0 = 0
                for i, tch in enumerate(sizes):
                    WT = wp_by_size[tch].tile([IN_D, tch * OUT_D], dt_mm)
                    weng = nc.sync if i % 2 == 0 else nc.scalar
                    weng.dma_start(
                        WT[:], wr[:, c0 * OUT_D:(c0 + tch) * OUT_D]
                    )
                    tiles.append((c0, c0 + tch, WT))
                    c0 += tch

                def w_slice(dl):
                    for a, b2, WT in tiles:
                        if a <= dl < b2:
                            return WT[:, (dl - a) * OUT_D:(dl - a + 1) * OUT_D]

                for g in range(N_BANKS):
                    PS = pp.tile([OUT_D, BANK_CH * B], F32)
                    if has_bias:
                        nc.tensor.matmul(
                            PS[:],
                            CB[:, g * OUT_D:(g + 1) * OUT_D],
                            CB[:, BN_COLS:CB_COLS],
                            start=True,
                            stop=False,
                        )
                    for j in range(BANK_CH):
                        dl = g * BANK_CH + j
                        nc.tensor.matmul(
                            PS[:, j * B:(j + 1) * B],
                            w_slice(dl),
                            XC[:, dl * B:(dl + 1) * B],
                            start=(not has_bias) and j == 0,
                            stop=(j == BANK_CH - 1),
                        )
                    OB = op.tile([OUT_D, BANK_CH * B], out_dt)
                    base = g * BANK_CH * B
                    nc.vector.tensor_copy(OB[:], PS[:])
                    # All outs on SWDGE, including the last ones: routing the
                    # final outs through the (idle) HWDGE rings was tried and
                    # regressed ~6 us.
                    nc.gpsimd.dma_start(
                        outr[:, base:base + BANK_CH * B], OB[:]
                    )
            else:
                nch = tile_ch * wcols_per_ch
                for t in range(n_tiles):
                    WT = wpa.tile([IN_D, nch], dt_mm)
                    # Alternate the two HWDGE rings so W transfers overlap.
                    weng = nc.sync if t % 2 == 0 else nc.scalar
                    weng.dma_start(WT[:], wr[:, t * nch:(t + 1) * nch])
                    for h in range(banks_per_tile):
                        g = t * banks_per_tile + h
                        PS = pp.tile([OUT_D, BANK_CH * B], F32)
                        if has_bias:
                            # Seed bank: PS[o, j*16+b] = bias[g*32+j, o].
                            nc.tensor.matmul(
                                PS[:],
                                CB[:, g * OUT_D:(g + 1) * OUT_D],
                                CB[:, BN_COLS:CB_COLS],
                                start=True,
                                stop=False,
                            )
                        for j in range(BANK_CH):
                            jt = h * BANK_CH + j
                            dl = g * BANK_CH + j
                            out_sl = PS[:, j * B:(j + 1) * B]
                            whi = WT[
                                :, jt * wcols_per_ch:jt * wcols_per_ch + OUT_D
                            ]
                            xhi = XC[:, dl * B:(dl + 1) * B]
                            nc.tensor.matmul(
                                out_sl,
                                whi,
                                xhi,
                                start=(not has_bias) and j == 0,
                                stop=(mode != "mix3") and (j == BANK_CH - 1),
                            )
                            if mode == "mix3":
                                wlo = WT[
                                    :,
                                    jt * wcols_per_ch + OUT_D:
                                    (jt + 1) * wcols_per_ch,
                                ]
                                xlo = XC[
                                    :, X_COLS + dl * B:X_COLS + (dl + 1) * B
                                ]
                                nc.tensor.matmul(
                                    out_sl, whi, xlo, start=False, stop=False
                                )
                                nc.tensor.matmul(
                                    out_sl,
                                    wlo,
                                    xhi,
                                    start=False,
                                    stop=(j == BANK_CH - 1),
                                )
                        OB = op.tile([OUT_D, BANK_CH * B], out_dt)
                        nc.vector.tensor_copy(OB[:], PS[:])
                        nc.sync.dma_start(
                            outr[:, g * BANK_CH * B:(g + 1) * BANK_CH * B],
                            OB[:],
                        )

    nc.finalize()
    return nc


def _pack_x(x, sl):
    # [b, i, dslab] -> [i, dl*16+b]
    return np.ascontiguousarray(x[:, :, sl].transpose(1, 2, 0)).reshape(
        IN_D, X_COLS
    )


def _pack_bias(b, sl, eh):
    bnr = np.ascontiguousarray(
        b[sl].reshape(N_BANKS, BANK_CH, OUT_D).transpose(1, 0, 2)
    ).reshape(BANK_CH, BN_COLS)
    cbv = np.zeros((BANK_CH, CB_COLS), dtype=ml_dtypes.bfloat16)
    cbv[:, :BN_COLS] = bnr.astype(ml_dtypes.bfloat16)
    cbv[:, BN_COLS:] = eh.astype(ml_dtypes.bfloat16)
    return cbv


def _prep_core_inputs(x, W, b, mode, has_bias):
    _, np_mm = _DT[mode]
    eh = np.repeat(np.eye(BANK_CH, dtype=np.float32), B, axis=1)
    if mode == "mix3":
        bf = ml_dtypes.bfloat16
        xh = x.astype(bf)
        xl = (x - xh.astype(np.float32)).astype(bf)
        Wh = W.astype(bf)
        Wl = (W - Wh.astype(np.float32)).astype(bf)
    in_maps = []
    for c in range(NCORES):
        sl = slice(c * D_C, (c + 1) * D_C)
        if mode == "mix3":
            xcv = np.concatenate(
                [_pack_x(xh.astype(np.float32), sl), _pack_x(xl.astype(np.float32), sl)],
                axis=1,
            ).astype(bf)
            wrv = np.ascontiguousarray(
                np.stack(
                    [Wh[sl].transpose(1, 0, 2), Wl[sl].transpose(1, 0, 2)],
                    axis=2,
                )
            ).reshape(IN_D, D_C * 2 * OUT_D)
            m = {"xc": xcv, "wr": wrv}
            if has_bias:
                m["cb"] = _pack_bias(b, sl, eh)
            in_maps.append(m)
            continue
        xr = _pack_x(x, sl).astype(np_mm, copy=False)
        wrv = (
            np.ascontiguousarray(W[sl].transpose(1, 0, 2))
            .reshape(IN_D, D_C * OUT_D)
            .astype(np_mm, copy=False)
        )
        m = {"xc": xr, "wr": wrv}
        if has_bias:
            m["cb"] = _pack_bias(b, sl, eh)
        in_maps.append(m)
    return in_maps


def run(inputs, trace=False, mode=None):
    mode = mode or MM_DTYPE
    x = np.asarray(inputs["x"], dtype=np.float32)
    W = np.asarray(inputs["W"], dtype=np.float32)
    b = np.asarray(inputs["b"], dtype=np.float32)
    has_bias = bool(np.any(b))
    key = (mode, has_bias)
    if key not in _cached:
        _cached[key] = _build(mode, has_bias)
    in_maps = _prep_core_inputs(x, W, b, mode, has_bias)
    res = run_bass_kernel_spmd(
        _cached[key], in_maps, core_ids=list(range(NCORES)), trace=trace
    )
    out = np.empty((B, OUT_D, D_TOTAL), dtype=np.float32)
    for c in range(NCORES):
        sl = slice(c * D_C, (c + 1) * D_C)
        out[:, :, sl] = (
            np.asarray(res.results[c]["outr"])
            .astype(np.float32)
            .reshape(OUT_D, D_C, B)
            .transpose(2, 0, 1)
        )
    return out, res


def kernel(**inputs):
    out, _ = run(inputs)
    return out



# revision 33
# speedup vs baseline: 1.0555x; 1.0352x over previous
"""Trainium2 Bass kernel for grouped per-channel linears (nn_GroupedLinearsAdvanced).

Math: out[b, o, d] = sum_i x[b, i, d] * W[d, i, o] + bias[d, o]
with x: [16, 128, 4096] f32, W: [4096, 128, 128] f32, bias: [4096, 128] f32,
out: [16, 128, 4096] f32.

Sharding: channel dim D=4096 split into 8 contiguous slabs of 512 channels,
one per NeuronCore; x slices replicated per-slab, no cross-device reduction.

Per-core dataflow (DMA-bound; ~20 MB of HBM traffic per core at ~26 GB/s
per SDMA engine x 16 engines):
  - host pre-permutes inputs so every DMA moves long contiguous
    per-partition runs; x and W are cast to bf16 on host (halves W traffic
    vs fp32; rel-err ~3e-3 against the fp32 reference),
  - x slab resident in SBUF: layout [i, dl*16+b]; both 1 MB chunks on the
    scalar HWDGE ring,
  - the whole 16 MB W slab is prefetched through SBUF in 8 x 2 MB tiles,
    all dma_starts issued up front, tiles alternating between the sync and
    scalar HWDGE rings (SDMA engines alternate descriptors between rings;
    a single ring cannot sustain line rate),
  - per channel: one matmul  PS[o, b] = W_d.T @ x_d.T  (lhsT = W_d),
    32 channels accumulate side-by-side into one 512-f32 PSUM bank,
  - bias (when nonzero) seeds each PSUM bank via a bf16 one-hot expansion
    matmul: PS[o, j*16+b] = bias[g*32+j, o] = (BN_g).T @ E; skipped when
    the supplied bias is identically zero,
  - DVE evacuates each bank to SBUF casting f32 -> bf16 (halves out DMA),
  - outs stream to HBM via the GPSIMD SWDGE queue so they never queue in
    a HWDGE ring FIFO behind undrained W packets; host casts back to f32.

MM_DTYPE picks the tensor-engine path for x/W:
  "f32"  — exact fp32 (hardware runs 2 half-speed passes per matmul),
  "f32r" — same fp32 bytes, single-pass reduced-precision PE mode,
  "bf16" — host-side cast, halves DMA traffic, single-pass matmuls + FWL,
  "mix3" — bf16 hi+lo split, fp32-class accuracy at fp32 bytes (slow path).
"""

import ml_dtypes
import numpy as np

from concourse import bacc, mybir, tile
from concourse.bass_utils import run_bass_kernel_spmd

B = 16           # batch
IN_D = 128       # contraction dim (SBUF partitions)
OUT_D = 128      # per-channel output dim
D_TOTAL = 4096   # channels
NCORES = 8
D_C = D_TOTAL // NCORES      # 512 channels per core
BANK_CH = 32                 # channels per PSUM bank (32*16 = 512 fp32 = 1 bank)
N_BANKS = D_C // BANK_CH     # 16

X_COLS = D_C * B                 # 8192
BN_COLS = N_BANKS * OUT_D        # 2048
EH_COLS = BANK_CH * B            # 512
CB_COLS = BN_COLS + EH_COLS      # bias + one-hot constant tensor

F32 = mybir.dt.float32
BF16 = mybir.dt.bfloat16

MM_DTYPE = "bf16"

_DT = {
    "f32": (F32, np.float32),
    "f32r": (mybir.dt.float32r, np.float32),
    "bf16": (BF16, ml_dtypes.bfloat16),
    # mix3: W and x split into bf16 hi+lo parts; 3 single-pass matmuls
    # per channel (hi*hi + lo*hi + hi*lo) recover ~1e-5 accuracy while
    # keeping bf16 tensor-engine throughput. Same HBM bytes as fp32.
    "mix3": (BF16, ml_dtypes.bfloat16),
}

_cached = {}


def _build(mode, has_bias):
    dt_mm, _ = _DT[mode]
    nparts = 2 if mode == "mix3" else 1  # hi/lo operand copies
    out_dt = BF16 if mode == "bf16" else F32
    nc = bacc.Bacc()
    xc = nc.dram_tensor("xc", [IN_D, nparts * X_COLS], dt_mm, kind="ExternalInput")
    wr = nc.dram_tensor(
        "wr", [IN_D, nparts * D_C * OUT_D], dt_mm, kind="ExternalInput"
    )
    if has_bias:
        cb = nc.dram_tensor("cb", [BANK_CH, CB_COLS], BF16, kind="ExternalInput")
    outr = nc.dram_tensor("outr", [OUT_D, D_C * B], out_dt, kind="ExternalOutput")

    # Streaming-path (non-prefetch) tile size; the prefetch path below uses
    # `sizes` instead.
    tile_ch = 32 // nparts
    n_tiles = D_C // tile_ch
    banks_per_tile = max(1, tile_ch // BANK_CH)
    wcols_per_ch = nparts * OUT_D
    # Prefetch the whole W slab when it fits in SBUF (bf16: 8 tiles x
    # 16 KB/partition = 128 KB). All W dma_starts issue up front, so a
    # later out-DMA's semaphore wait on the sync ring can never delay a
    # W transfer (HWDGE rings are FIFO per issuing engine).
    w_kb_per_part = n_tiles * tile_ch * wcols_per_ch * (2 if dt_mm != F32 else 4)
    prefetch_all = w_kb_per_part <= 128 * 1024
    # Uniform 64-channel (2 MB) W tiles. Finer or variable tail tiles were
    # tried and REGRESSED: at stream end the per-transfer completion
    # semaphores fire 1-3 us after the data (3-way ring contention for the
    # final sem descriptors), and the in-order PE serializes those waits —
    # fewer, uniform transfers win. Splitting the final bank's cast/store
    # also regressed (extra tail dependencies).
    sizes = [2 * BANK_CH] * (N_BANKS // 2)
    with tile.TileContext(nc) as tc:
        with (
            tc.tile_pool(name="xp", bufs=1) as xp,
            tc.tile_pool(name="wpa", bufs=len(sizes)) as wpa,
            tc.tile_pool(name="op", bufs=N_BANKS) as op,
            tc.tile_pool(name="pp", bufs=8, space="PSUM") as pp,
        ):
            wp_by_size = {2 * BANK_CH: wpa}
            XC = xp.tile([IN_D, nparts * X_COLS], dt_mm)
            # Chunks so early banks can start before the back half lands;
            # chunk-major order so bank 0 gets hi AND lo slices first.
            # One chunk per ring keeps ring byte loads equal.
            # One chunk per HWDGE ring: with both chunks on one ring that
            # ring carries 2 MB more W+x than the other and its final tiles
            # land ~9 us later, pacing the whole tail (trace-verified).
            # Loading x via the SWDGE queue instead was tried and regressed
            # ~5 us (every matmul waits on x; SWDGE completion is slower).
            half = X_COLS // 2
            for ch in range(2):
                for p in range(nparts):
                    lo = p * X_COLS + ch * half
                    xeng = nc.sync if (ch + p) % 2 == 0 else nc.scalar
                    xeng.dma_start(
                        XC[:, lo:lo + half], xc[:, lo:lo + half]
                    )
            if has_bias:
                CB = xp.tile([BANK_CH, CB_COLS], BF16)
                nc.scalar.dma_start(CB[:], cb[:])

            if prefetch_all:
                # W alternates across both HWDGE rings (one ring alone cannot
                # sustain full line rate; SDMA engines alternate descriptors
                # between rings, so both rings must carry work). Outs go to
                # the GPSIMD SWDGE queue: in a HWDGE ring they would sit in
                # FIFO order behind the remaining W packets and only drain
                # after the whole W stream.
                tiles = []
                c0 = 0
                for i, tch in enumerate(sizes):
                    WT = wp_by_size[tch].tile([IN_D, tch * OUT_D], dt_mm)
                    weng = nc.sync if i % 2 == 0 else nc.scalar
                    weng.dma_start(
                        WT[:], wr[:, c0 * OUT_D:(c0 + tch) * OUT_D]
                    )
                    tiles.append((c0, c0 + tch, WT))
                    c0 += tch

                def w_slice(dl):
                    for a, b2, WT in tiles:
                        if a <= dl < b2:
                            return WT[:, (dl - a) * OUT_D:(dl - a + 1) * OUT_D]

                for g in range(N_BANKS):
                    PS = pp.tile([OUT_D, BANK_CH * B], F32)
                    if has_bias:
                        nc.tensor.matmul(
                            PS[:],
                            CB[:, g * OUT_D:(g + 1) * OUT_D],
                            CB[:, BN_COLS:CB_COLS],
                            start=True,
                            stop=False,
                        )
                    for j in range(BANK_CH):
                        dl = g * BANK_CH + j
                        nc.tensor.matmul(
                            PS[:, j * B:(j + 1) * B],
                            w_slice(dl),
                            XC[:, dl * B:(dl + 1) * B],
                            start=(not has_bias) and j == 0,
                            stop=(j == BANK_CH - 1),
                        )
                    OB = op.tile([OUT_D, BANK_CH * B], out_dt)
                    base = g * BANK_CH * B
                    nc.vector.tensor_copy(OB[:], PS[:])
                    # All outs on SWDGE, including the last ones: routing the
                    # final outs through the (idle) HWDGE rings was tried and
                    # regressed ~6 us.
                    nc.gpsimd.dma_start(
                        outr[:, base:base + BANK_CH * B], OB[:]
                    )
            else:
                nch = tile_ch * wcols_per_ch
                for t in range(n_tiles):
                    WT = wpa.tile([IN_D, nch], dt_mm)
                    # Alternate the two HWDGE rings so W transfers overlap.
                    weng = nc.sync if t % 2 == 0 else nc.scalar
                    weng.dma_start(WT[:], wr[:, t * nch:(t + 1) * nch])
                    for h in range(banks_per_tile):
                        g = t * banks_per_tile + h
                        PS = pp.tile([OUT_D, BANK_CH * B], F32)
                        if has_bias:
                            # Seed bank: PS[o, j*16+b] = bias[g*32+j, o].
                            nc.tensor.matmul(
                                PS[:],
                                CB[:, g * OUT_D:(g + 1) * OUT_D],
                                CB[:, BN_COLS:CB_COLS],
                                start=True,
                                stop=False,
                            )
                        for j in range(BANK_CH):
                            jt = h * BANK_CH + j
                            dl = g * BANK_CH + j
                            out_sl = PS[:, j * B:(j + 1) * B]
                            whi = WT[
                                :, jt * wcols_per_ch:jt * wcols_per_ch + OUT_D
                            ]
                            xhi = XC[:, dl * B:(dl + 1) * B]
                            nc.tensor.matmul(
                                out_sl,
                                whi,
                                xhi,
                                start=(not has_bias) and j == 0,
                                stop=(mode != "mix3") and (j == BANK_CH - 1),
                            )
                            if mode == "mix3":
                                wlo = WT[
                                    :,
                                    jt * wcols_per_ch + OUT_D:
                                    (jt + 1) * wcols_per_ch,
                                ]
                                xlo = XC[
                                    :, X_COLS + dl * B:X_COLS + (dl + 1) * B
                                ]
                                nc.tensor.matmul(
                                    out_sl, whi, xlo, start=False, stop=False
                                )
                                nc.tensor.matmul(
                                    out_sl,
                                    wlo,
                                    xhi,
                                    start=False,
                                    stop=(j == BANK_CH - 1),
                                )
                        OB = op.tile([OUT_D, BANK_CH * B], out_dt)
                        nc.vector.tensor_copy(OB[:], PS[:])
                        nc.sync.dma_start(
                            outr[:, g * BANK_CH * B:(g + 1) * BANK_CH * B],
                            OB[:],
                        )

    nc.finalize()
    return nc


def _pack_x(x, sl):
    # [b, i, dslab] -> [i, dl*16+b]
    return np.ascontiguousarray(x[:, :, sl].transpose(1, 2, 0)).reshape(
        IN_D, X_COLS
    )


def _pack_bias(b, sl, eh):
    bnr = np.ascontiguousarray(
        b[sl].reshape(N_BANKS, BANK_CH, OUT_D).transpose(1, 0, 2)
    ).reshape(BANK_CH, BN_COLS)
    cbv = np.zeros((BANK_CH, CB_COLS), dtype=ml_dtypes.bfloat16)
    cbv[:, :BN_COLS] = bnr.astype(ml_dtypes.bfloat16)
    cbv[:, BN_COLS:] = eh.astype(ml_dtypes.bfloat16)
    return cbv


def _prep_core_inputs(x, W, b, mode, has_bias):
    _, np_mm = _DT[mode]
    eh = np.repeat(np.eye(BANK_CH, dtype=np.float32), B, axis=1)
    if mode == "mix3":
        bf = ml_dtypes.bfloat16
        xh = x.astype(bf)
        xl = (x - xh.astype(np.float32)).astype(bf)
        Wh = W.astype(bf)
        Wl = (W - Wh.astype(np.float32)).astype(bf)
    in_maps = []
    for c in range(NCORES):
        sl = slice(c * D_C, (c + 1) * D_C)
        if mode == "mix3":
            xcv = np.concatenate(
                [_pack_x(xh.astype(np.float32), sl), _pack_x(xl.astype(np.float32), sl)],
                axis=1,
            ).astype(bf)
            wrv = np.ascontiguousarray(
                np.stack(
                    [Wh[sl].transpose(1, 0, 2), Wl[sl].transpose(1, 0, 2)],
                    axis=2,
                )
            ).reshape(IN_D, D_C * 2 * OUT_D)
            m = {"xc": xcv, "wr": wrv}
            if has_bias:
                m["cb"] = _pack_bias(b, sl, eh)
            in_maps.append(m)
            continue
        xr = _pack_x(x, sl).astype(np_mm, copy=False)
        wrv = (
            np.ascontiguousarray(W[sl].transpose(1, 0, 2))
            .reshape(IN_D, D_C * OUT_D)
            .astype(np_mm, copy=False)
        )
        m = {"xc": xr, "wr": wrv}
        if has_bias:
            m["cb"] = _pack_bias(b, sl, eh)
        in_maps.append(m)
    return in_maps


def run(inputs, trace=False, mode=None):
    mode = mode or MM_DTYPE
    x = np.asarray(inputs["x"], dtype=np.float32)
    W = np.asarray(inputs["W"], dtype=np.float32)
    b = np.asarray(inputs["b"], dtype=np.float32)
    has_bias = bool(np.any(b))
    key = (mode, has_bias)
    if key not in _cached:
        _cached[key] = _build(mode, has_bias)
    in_maps = _prep_core_inputs(x, W, b, mode, has_bias)
    res = run_bass_kernel_spmd(
        _cached[key], in_maps, core_ids=list(range(NCORES)), trace=trace
    )
    out = np.empty((B, OUT_D, D_TOTAL), dtype=np.float32)
    for c in range(NCORES):
        sl = slice(c * D_C, (c + 1) * D_C)
        out[:, :, sl] = (
            np.asarray(res.results[c]["outr"])
            .astype(np.float32)
            .reshape(OUT_D, D_C, B)
            .transpose(2, 0, 1)
        )
    return out, res


def kernel(**inputs):
    out, _ = run(inputs)
    return out

